# revision 26
# baseline (speedup 1.0000x reference)
"""Trainium2 Bass kernel for nn_Detector_54365696032903 (moe_routing).

Sharding: 8 cores = 4 batches x 2 expert-halves. Core c handles batch b=c//2,
experts l in [12*(c%2), 12*(c%2)+12). Each (b,l) "stream" runs the shared MLP,
its group MLP, and the time-attention-pool fully on-core (feature-major bf16
matmuls, LN affines folded into weights host-side). The per-expert router is a
sigmoid (no cross-expert softmax), so the only cross-core communication is a
2-rank AllReduce per (b)-pair of the fusion and shared partial sums, after
which both cores of the pair redundantly compute the tail (fusion LN+mean,
attn-pool of shared_mean, final LNs, classifier).
"""
import sys

sys.path.insert(0, "/opt/trn_rl_repo")

import numpy as np
import ml_dtypes

B, L, T, D = 4, 24, 400, 1024
H = 512
G = 4
HEADS = 2
DH = H // HEADS  # 256
TEMP = 1.75
ALPHA = 0.5
EPS = 1e-5
NCORE = 8
LPC = 12          # experts per core
NSTREAM = 13      # 12 expert streams + 1 time_ctx stream
TCH = [(0, 128), (128, 128), (256, 128), (384, 16)]  # token chunks of T=400
KD = D // 128     # 8 k-chunks of D
KH = H // 128     # 4 k-chunks of H

_BF = ml_dtypes.bfloat16


def _build_nc():
    import concourse.bass as bass
    import concourse.tile as tile
    from concourse import bacc, mybir

    f32 = mybir.dt.float32
    bf16 = mybir.dt.bfloat16
    AF = mybir.ActivationFunctionType
    AL = mybir.AluOpType
    AX = mybir.AxisListType

    nc = bacc.Bacc("TRN2", num_devices=NCORE)

    # ---------------- DRAM I/O ----------------
    x_d = nc.dram_tensor("x", [LPC, T, D], f32, kind="ExternalInput")
    alpha_d = nc.dram_tensor("alpha", [LPC], f32, kind="ExternalInput")

    ws1_d = nc.dram_tensor("ws1", [128, KD, H], bf16, kind="ExternalInput")
    bs1_d = nc.dram_tensor("bs1", [128, KH], f32, kind="ExternalInput")
    ws2_d = nc.dram_tensor("ws2", [128, KH, H], bf16, kind="ExternalInput")
    bs2_d = nc.dram_tensor("bs2", [128, KH], f32, kind="ExternalInput")
    wg1_d = nc.dram_tensor("wg1", [128, 2, KD, H], bf16, kind="ExternalInput")
    bg1_d = nc.dram_tensor("bg1", [128, 2, KH], f32, kind="ExternalInput")
    wg2_d = nc.dram_tensor("wg2", [128, 2, KH, H], bf16, kind="ExternalInput")
    b2r_d = nc.dram_tensor("b2r", [1, 2, H], bf16, kind="ExternalInput")  # g_b2 rows

    wqk_d = nc.dram_tensor("wqk", [128, KH, 2 * H], bf16, kind="ExternalInput")
    bqk_d = nc.dram_tensor("bqk", [128, 2 * KH], f32, kind="ExternalInput")
    wv_d = nc.dram_tensor("wv", [128, KH, H], bf16, kind="ExternalInput")
    bvr_d = nc.dram_tensor("bvr", [1, H], bf16, kind="ExternalInput")       # v bias row
    wo_d = nc.dram_tensor("wo", [128, KH, H], f32, kind="ExternalInput")
    bo_d = nc.dram_tensor("bo", [128, KH], f32, kind="ExternalInput")

    rw1_d = nc.dram_tensor("rw1", [128, KH, 128], f32, kind="ExternalInput")
    rb1_d = nc.dram_tensor("rb1", [128, 1], f32, kind="ExternalInput")
    rw2_d = nc.dram_tensor("rw2", [128, 1], f32, kind="ExternalInput")
    rb2t_d = nc.dram_tensor("rb2t", [1, 1], f32, kind="ExternalInput")  # rb2/(2*TEMP)

    png_d = nc.dram_tensor("png", [128, KH], f32, kind="ExternalInput")
    pnb_d = nc.dram_tensor("pnb", [128, KH], f32, kind="ExternalInput")
    pnfg_d = nc.dram_tensor("pnfg", [128, KH], f32, kind="ExternalInput")
    pnfb_d = nc.dram_tensor("pnfb", [128, KH], f32, kind="ExternalInput")

    cw1_d = nc.dram_tensor("cw1", [128, KH, 256], f32, kind="ExternalInput")
    cb1_d = nc.dram_tensor("cb1", [128, 2], f32, kind="ExternalInput")
    cw2_d = nc.dram_tensor("cw2", [128, 2, 128], f32, kind="ExternalInput")
    cb2_d = nc.dram_tensor("cb2", [128, 1], f32, kind="ExternalInput")
    cw3_d = nc.dram_tensor("cw3", [128, 2], f32, kind="ExternalInput")
    cb3_d = nc.dram_tensor("cb3", [2, 1], f32, kind="ExternalInput")

    tp_out_d = nc.dram_tensor("tp_out", [LPC, H], f32, kind="ExternalOutput")
    rw_out_d = nc.dram_tensor("rw_out", [1, LPC], f32, kind="ExternalOutput")
    fa_out_d = nc.dram_tensor("fa_out", [128, KH], f32, kind="ExternalOutput")
    lg_out_d = nc.dram_tensor("lg_out", [2, 1], f32, kind="ExternalOutput")

    with tile.TileContext(nc) as tc:
        with (
            tc.tile_pool(name="wts", bufs=1) as wts,
            tc.tile_pool(name="state", bufs=1) as st,
            tc.tile_pool(name="sp", bufs=2) as sp,
            tc.tile_pool(name="psum", bufs=2, space="PSUM") as ps,
            tc.tile_pool(name="dram", bufs=1, space="DRAM") as dr,
        ):
            # ---------- constants ----------
            ones_col = wts.tile([128, 1], bf16, name="ones_col")
            nc.vector.memset(ones_col, 1.0)
            ones_rowb = wts.tile([1, 128], bf16, name="ones_rowb")
            nc.vector.memset(ones_rowb, 1.0)
            ones_rowf = wts.tile([1, 128], f32, name="ones_rowf")
            nc.vector.memset(ones_rowf, 1.0)
            ones_w = wts.tile([1, T], bf16, name="ones_w")
            nc.vector.memset(ones_w, 1.0)
            epsP = wts.tile([128, 1], f32, name="epsP")
            nc.vector.memset(epsP, EPS)
            eps1 = wts.tile([1, 1], f32, name="eps1")
            nc.vector.memset(eps1, EPS)

            def load(d, name):
                t = wts.tile(list(d.shape), d.dtype, name=name)
                nc.sync.dma_start(t[:], d[:])
                return t

            ws1 = load(ws1_d, "ws1")
            bs1 = load(bs1_d, "bs1")
            ws2 = load(ws2_d, "ws2")
            bs2 = load(bs2_d, "bs2")
            wg1 = load(wg1_d, "wg1")
            bg1 = load(bg1_d, "bg1")
            wg2 = load(wg2_d, "wg2")
            b2r = load(b2r_d, "b2r")
            wqk = load(wqk_d, "wqk")
            bqk = load(bqk_d, "bqk")
            wv = load(wv_d, "wv")
            bvr = load(bvr_d, "bvr")
            wo = load(wo_d, "wo")
            bo = load(bo_d, "bo")
            rw1 = load(rw1_d, "rw1")
            rb1 = load(rb1_d, "rb1")
            rw2 = load(rw2_d, "rw2")
            rb2t = load(rb2t_d, "rb2t")
            png = load(png_d, "png")
            pnb = load(pnb_d, "pnb")
            pnfg = load(pnfg_d, "pnfg")
            pnfb = load(pnfb_d, "pnfb")
            cw1 = load(cw1_d, "cw1")
            cb1 = load(cb1_d, "cb1")
            cw2 = load(cw2_d, "cw2")
            cb2 = load(cb2_d, "cb2")
            cw3 = load(cw3_d, "cw3")
            cb3 = load(cb3_d, "cb3")

            # per-stream alpha broadcast [128, LPC]
            alB = wts.tile([128, LPC], f32, name="alB")
            _aap = alpha_d[:]
            nc.sync.dma_start(
                alB[:],
                bass.AP(tensor=_aap.tensor, offset=_aap.offset,
                        ap=[[0, 128]] + [list(p) for p in _aap.ap]),
            )

            # ---------- persistent state ----------
            fus_acc = st.tile([128, KH, T], f32, name="fus_acc")
            sh_acc = st.tile([128, KH, T], f32, name="sh_acc")
            nc.gpsimd.memset(fus_acc[:], 0.0)
            nc.gpsimd.memset(sh_acc[:], 0.0)
            tpT = st.tile([128, KH * NSTREAM], f32, name="tpT")
            osum_all = st.tile([128, KH * NSTREAM], f32, name="osum_all")
            rw_row = st.tile([1, LPC], f32, name="rw_row")

            # =========================================================
            # helpers
            # =========================================================
            def prep_stream(l):
                """Load x[l], LN-core (x-mu)*rstd in token-major, then
                DRAM-bounce XBAR DMA-transpose to feature-major xcT."""
                xcd = dr.tile([T, D], bf16, name=f"xcd_{l}", tag="xcd", bufs=2)
                for ti, (t0, p) in enumerate(TCH):
                    xt = sp.tile([128, D], f32, name=f"xt_{l}_{ti}", tag="xt", bufs=2)
                    nc.sync.dma_start(xt[:p], x_d[l, t0:t0 + p, :])
                    stats = sp.tile([128, 2, 6], f32, name=f"bnst_{l}_{ti}",
                                    tag="bnst", bufs=3)
                    nc.vector.bn_stats(stats[:p, 0], xt[:p, 0:512])
                    nc.vector.bn_stats(stats[:p, 1], xt[:p, 512:1024])
                    mv = sp.tile([128, 2], f32, name=f"mv_{l}_{ti}", tag="mv", bufs=3)
                    nc.vector.bn_aggr(mv[:p], stats[:p])
                    rstd = sp.tile([128, 1], f32, name=f"rstd_{l}_{ti}", tag="rstd",
                                   bufs=3)
                    nc.scalar.activation(rstd[:p], mv[:p, 1:2], AF.Ln, bias=epsP[:p])
                    nc.scalar.activation(rstd[:p], rstd[:p], AF.Exp, scale=-0.5)
                    xc = sp.tile([128, D], bf16, name=f"xc_{l}_{ti}", tag="xc", bufs=3)
                    nc.gpsimd.tensor_scalar(
                        xc[:p], xt[:p], scalar1=mv[:p, 0:1], scalar2=rstd[:p],
                        op0=AL.subtract, op1=AL.mult)
                    nc.sync.dma_start(xcd[t0:t0 + p, :], xc[:p])
                xcT = sp.tile([128, KD, T], bf16, name=f"xcT_{l}", tag="xcT", bufs=1)
                nc.sync.dma_start_transpose(xcT[:], xcd[:])
                return xcT

            def mlp1(l, xcT, w1, b1, tagp):
                """Layer 1: GELU(W1^T xc + b1), feature-major bf16 out."""
                h1 = sp.tile([128, KH, T], bf16, name=f"h1{tagp}_{l}", tag=f"h1{tagp}", bufs=1)
                for m in range(KH):
                    mm = ps.tile([128, T], f32, name=f"mm1{tagp}_{l}_{m}", tag="mm400")
                    for k in range(KD):
                        nc.tensor.matmul(mm[:], w1[:, k, m * 128:(m + 1) * 128],
                                         xcT[:, k, :], start=(k == 0),
                                         stop=(k == KD - 1))
                    nc.scalar.activation(h1[:, m, :], mm[:], AF.Gelu,
                                         bias=b1[:, m:m + 1])
                return h1

            def pool_stats(l, S):
                """B1: LN stats over features via PE colsums -> mur row +
                rstd partition-broadcast."""
                sfx = f"p{l}"
                w_bf = S["w"]
                sq = sp.tile([128, KH, T], bf16, name=f"sq_{sfx}", tag="sq", bufs=1)
                nc.vector.tensor_tensor(sq[:], w_bf[:], w_bf[:], op=AL.mult)
                s_ps = ps.tile([1, T], f32, name=f"sps_{sfx}", tag="row")
                q_ps = ps.tile([1, T], f32, name=f"qps_{sfx}", tag="row")
                for c in range(KH):
                    nc.tensor.matmul(s_ps[:], ones_col[:], w_bf[:, c, :],
                                     start=(c == 0), stop=(c == KH - 1))
                for c in range(KH):
                    nc.tensor.matmul(q_ps[:], ones_col[:], sq[:, c, :],
                                     start=(c == 0), stop=(c == KH - 1))
                mu_row = sp.tile([1, T], f32, name=f"mu_{sfx}", tag="mu_row", bufs=2)
                nc.scalar.activation(mu_row[:], s_ps[:], AF.Copy, scale=1.0 / H)
                e2_row = sp.tile([1, T], f32, name=f"e2_{sfx}", tag="e2_row", bufs=2)
                nc.scalar.activation(e2_row[:], q_ps[:], AF.Copy, scale=1.0 / H)
                var_row = sp.tile([1, T], f32, name=f"var_{sfx}", tag="var_row", bufs=2)
                nc.vector.tensor_tensor(var_row[:], mu_row[:], mu_row[:], op=AL.mult)
                nc.vector.tensor_tensor(var_row[:], e2_row[:], var_row[:],
                                        op=AL.subtract)
                nc.scalar.activation(var_row[:], var_row[:], AF.Ln, bias=eps1[:])
                rstd_bf = sp.tile([1, T], bf16, name=f"rstdb_{sfx}", tag="rstd_bf",
                                  bufs=2)
                nc.scalar.activation(rstd_bf[:], var_row[:], AF.Exp, scale=-0.5)
                mu_bf = sp.tile([1, T], bf16, name=f"mubf_{sfx}", tag="mur_bf", bufs=2)
                nc.any.tensor_copy(mu_bf[:], mu_row[:])
                rb_ps = ps.tile([128, T], f32, name=f"rbps_{sfx}", tag="mm400")
                nc.tensor.matmul(rb_ps[:], ones_rowb[:], rstd_bf[:], start=True,
                                 stop=True)
                rstdB = sp.tile([128, T], bf16, name=f"rstdB_{sfx}", tag="rstdB")
                nc.any.tensor_copy(rstdB[:], rb_ps[:])
                mB_ps = ps.tile([128, T], f32, name=f"mBps_{sfx}", tag="mm400")
                nc.tensor.matmul(mB_ps[:], ones_rowb[:], mu_bf[:], start=True,
                                 stop=True)
                muB = sp.tile([128, T], bf16, name=f"muB_{sfx}", tag="muB")
                nc.any.tensor_copy(muB[:], mB_ps[:])
                S["muB"] = muB
                S["rstdB"] = rstdB

            def pool_qkv(l, S):
                """B2: wn = (w-muB)*rstdB; qk (feature-major), v (token-major)."""
                sfx = f"p{l}"
                w_bf, muB, rstdB = S["w"], S["muB"], S["rstdB"]
                wn = sp.tile([128, KH, T], bf16, name=f"wn_{sfx}", tag="wr", bufs=1)
                nc.vector.tensor_tensor(
                    wn[:], w_bf[:], muB[:, None, :].to_broadcast((128, KH, T)),
                    op=AL.subtract)
                nc.vector.tensor_tensor(
                    wn[:], wn[:], rstdB[:, None, :].to_broadcast((128, KH, T)),
                    op=AL.mult)
                qkT = sp.tile([128, 2 * KH, T], bf16, name=f"qkT_{sfx}", tag="qkT",
                              bufs=1)
                for m in range(2 * KH):
                    mm = ps.tile([128, T], f32, name=f"qk_{sfx}_{m}", tag="mm400")
                    for k in range(KH):
                        nc.tensor.matmul(mm[:], wqk[:, k, m * 128:(m + 1) * 128],
                                         wn[:, k, :], start=(k == 0),
                                         stop=(k == KH - 1))
                    nc.scalar.activation(qkT[:, m, :], mm[:], AF.Identity,
                                         bias=bqk[:, m:m + 1])
                v_sb = sp.tile([128, len(TCH), H], bf16, name=f"v_{sfx}", tag="v_sb",
                               bufs=1)
                for ti, (t0, p) in enumerate(TCH):
                    mm = ps.tile([128, H], f32, name=f"vp_{sfx}_{ti}", tag="mm512")
                    for k in range(KH):
                        nc.tensor.matmul(mm[:p], wn[:, k, t0:t0 + p], wv[:, k, :],
                                         start=(k == 0), stop=False)
                    nc.tensor.matmul(mm[:p], ones_w[0:1, 0:p], bvr[0:1, :],
                                     start=False, stop=True)
                    nc.any.tensor_copy(v_sb[:p, ti, :], mm[:p])
                S["qkT"] = qkT
                S["v"] = v_sb

            def pool_scores(l, S):
                """B3: scores -> exp -> denominators -> 1/den broadcast."""
                sfx = f"p{l}"
                qkT = S["qkT"]
                exps = sp.tile([128, HEADS, len(TCH), T], bf16,
                               name=f"exps_{sfx}", tag="exps", bufs=1)
                invBs = []
                for h in range(HEADS):
                    for ti, (t0, p) in enumerate(TCH):
                        mm = ps.tile([128, T], f32, name=f"sc_{sfx}_{h}_{ti}",
                                     tag="mm400")
                        for kc in range(2):
                            nc.tensor.matmul(
                                mm[:p], qkT[:, KH + 2 * h + kc, t0:t0 + p],
                                qkT[:, 2 * h + kc, :],
                                start=(kc == 0), stop=(kc == 1))
                        nc.scalar.activation(exps[:p, h, ti, :], mm[:p], AF.Exp,
                                             scale=float(1.0 / np.sqrt(DH)))
                    d_ps = ps.tile([1, T], f32, name=f"den_{sfx}_{h}", tag="row")
                    for ti, (t0, p) in enumerate(TCH):
                        nc.tensor.matmul(d_ps[:], ones_col[:p], exps[:p, h, ti, :],
                                         start=(ti == 0), stop=(ti == len(TCH) - 1))
                    inv_row = sp.tile([1, T], f32, name=f"inv_{sfx}_{h}",
                                      tag="inv_row", bufs=2)
                    nc.vector.reciprocal(inv_row[:], d_ps[:])
                    iB_ps = ps.tile([128, T], f32, name=f"ibps_{sfx}_{h}", tag="mm400")
                    nc.tensor.matmul(iB_ps[:], ones_rowf[:], inv_row[:], start=True,
                                     stop=True)
                    invB = sp.tile([128, T], f32, name=f"invB_{sfx}_{h}", tag="invB")
                    nc.any.tensor_copy(invB[:], iB_ps[:])
                    invBs.append(invB)
                S["exps"] = exps
                S["invB"] = invBs

            def pool_av(l, S):
                """B4: attention-weighted V, normalize + time-sum into
                osum_all[:, c*NSTREAM+l] (out_proj is batched at the end)."""
                sfx = f"p{l}"
                v_sb, exps, invBs = S["v"], S["exps"], S["invB"]
                for h in range(HEADS):
                    for m2 in range(2):
                        mm = ps.tile([128, T], f32, name=f"av_{sfx}_{h}_{m2}",
                                     tag="mm400")
                        for ti, (t0, p) in enumerate(TCH):
                            nc.tensor.matmul(
                                mm[:],
                                v_sb[:p, ti, h * DH + m2 * 128: h * DH + (m2 + 1) * 128],
                                exps[:p, h, ti, :],
                                start=(ti == 0), stop=(ti == len(TCH) - 1))
                        scr = sp.tile([128, T], bf16, name=f"scr_{sfx}_{h}_{m2}",
                                      tag="scr")
                        c = 2 * h + m2
                        nc.vector.scalar_tensor_tensor(
                            scr[:], mm[:], 1.0, invBs[h][:],
                            op0=AL.mult, op1=AL.mult,
                            accum_out=osum_all[:, c * NSTREAM + l:
                                               c * NSTREAM + l + 1])

            def router(l, weighted):
                """Router for stream l (GELU on ACT — emitted while the gelu
                table is loaded). Updates rw_row[:, l], fus_acc += rw*weighted."""
                sfx = f"r{l}"
                mm = ps.tile([128, 1], f32, name=f"r1_{sfx}", tag="row")
                for k in range(KH):
                    nc.tensor.matmul(mm[:], rw1[:, k, :],
                                     osum_all[:, k * NSTREAM + l:
                                              k * NSTREAM + l + 1],
                                     start=(k == 0), stop=(k == KH - 1))
                r1 = sp.tile([128, 1], f32, name=f"r1s_{sfx}", tag="r1", bufs=3)
                nc.scalar.activation(r1[:], mm[:], AF.Gelu, bias=rb1[:])
                lg = ps.tile([1, 1], f32, name=f"lg_{sfx}", tag="row")
                nc.tensor.matmul(lg[:], rw2[:], r1[:], start=True, stop=True)
                # sigmoid((z+rb2)/TEMP) = 0.5 + 0.5*tanh((z+rb2)/(2*TEMP))
                th = sp.tile([1, 1], f32, name=f"th_{sfx}", tag="th", bufs=3)
                nc.scalar.activation(th[:], lg[:], AF.Tanh,
                                     scale=1.0 / (2.0 * TEMP), bias=rb2t[:])
                nc.scalar.activation(rw_row[:, l:l + 1], th[:], AF.Copy,
                                     scale=ALPHA / 2.0,
                                     bias=ALPHA / 2.0 + (1.0 - ALPHA) / L)
                rwb_ps = ps.tile([128, 1], f32, name=f"rwb_{sfx}", tag="row")
                nc.tensor.matmul(rwb_ps[:], ones_rowf[:], rw_row[:, l:l + 1],
                                 start=True, stop=True)
                rwB = sp.tile([128, 1], f32, name=f"rwB_{sfx}", tag="rwB", bufs=3)
                nc.vector.tensor_copy(rwB[:], rwb_ps[:])
                nc.vector.scalar_tensor_tensor(
                    fus_acc[:], weighted[:], rwB[:], fus_acc[:],
                    op0=AL.mult, op1=AL.add)

            def ln_vec(in_col, gcol, bcol, sfx):
                """LN over 512 features stored feature-major as [128, KH] f32."""
                in_bf = sp.tile([128, KH], bf16, name=f"lvb_{sfx}", tag="lv_bf",
                                bufs=3)
                nc.vector.tensor_copy(in_bf[:], in_col[:])
                s_ps = ps.tile([1, 1], f32, name=f"lvs_{sfx}", tag="row")
                q_ps = ps.tile([1, 1], f32, name=f"lvq_{sfx}", tag="row")
                for c in range(KH):
                    nc.tensor.matmul(s_ps[:], in_bf[:, c:c + 1], ones_col[:],
                                     start=(c == 0), stop=(c == KH - 1))
                for c in range(KH):
                    nc.tensor.matmul(q_ps[:], in_bf[:, c:c + 1], in_bf[:, c:c + 1],
                                     start=(c == 0), stop=(c == KH - 1))
                mu1 = sp.tile([1, 1], f32, name=f"lvmu_{sfx}", tag="lv_mu", bufs=3)
                nc.scalar.activation(mu1[:], s_ps[:], AF.Copy, scale=1.0 / H)
                e2 = sp.tile([1, 1], f32, name=f"lve2_{sfx}", tag="lv_e2", bufs=3)
                nc.scalar.activation(e2[:], q_ps[:], AF.Copy, scale=1.0 / H)
                var = sp.tile([1, 1], f32, name=f"lvvar_{sfx}", tag="lv_var", bufs=3)
                nc.vector.tensor_tensor(var[:], mu1[:], mu1[:], op=AL.mult)
                nc.vector.tensor_tensor(var[:], e2[:], var[:], op=AL.subtract)
                nc.scalar.activation(var[:], var[:], AF.Ln, bias=eps1[:])
                rstd1 = sp.tile([1, 1], f32, name=f"lvrs_{sfx}", tag="lv_rstd", bufs=3)
                nc.scalar.activation(rstd1[:], var[:], AF.Exp, scale=-0.5)
                muB_ps = ps.tile([128, 1], f32, name=f"lvmB_{sfx}", tag="row")
                nc.tensor.matmul(muB_ps[:], ones_rowf[:], mu1[:], start=True,
                                 stop=True)
                rsB_ps = ps.tile([128, 1], f32, name=f"lvrB_{sfx}", tag="row")
                nc.tensor.matmul(rsB_ps[:], ones_rowf[:], rstd1[:], start=True,
                                 stop=True)
                muB = sp.tile([128, 1], f32, name=f"lvmBs_{sfx}", tag="lv_muB", bufs=3)
                nc.vector.tensor_copy(muB[:], muB_ps[:])
                rsB = sp.tile([128, 1], f32, name=f"lvrBs_{sfx}", tag="lv_rsB", bufs=3)
                nc.vector.tensor_copy(rsB[:], rsB_ps[:])
                out = sp.tile([128, KH], f32, name=f"lvout_{sfx}", tag="lv_out",
                              bufs=3)
                nc.vector.tensor_scalar(out[:], in_col[:], scalar1=muB[:],
                                        scalar2=rsB[:], op0=AL.subtract, op1=AL.mult)
                nc.vector.tensor_tensor(out[:], out[:], gcol[:], op=AL.mult)
                nc.vector.tensor_tensor(out[:], out[:], bcol[:], op=AL.add)
                return out

            def mlp2_weighted(l):
                """Layer-2 of both MLPs + weighted combine + shared accumulate."""
                gi = l // 6
                h1s, h1g = HS[l]
                sharedT = sp.tile([128, KH, T], bf16, name=f"shT_{l}",
                                  tag="sharedT", bufs=1)
                weighted = sp.tile([128, KH, T], bf16, name=f"wt_{l}",
                                   tag="weighted", bufs=3)
                for m in range(KH):
                    smm = ps.tile([128, T], f32, name=f"mm2s_{l}_{m}", tag="mm400")
                    for k in range(KH):
                        nc.tensor.matmul(smm[:], ws2[:, k, m * 128:(m + 1) * 128],
                                         h1s[:, k, :], start=(k == 0),
                                         stop=(k == KH - 1))
                    nc.scalar.activation(sharedT[:, m, :], smm[:], AF.Identity,
                                         bias=bs2[:, m:m + 1])
                    gmm = ps.tile([128, T], f32, name=f"mm2g_{l}_{m}", tag="mm400")
                    for k in range(KH):
                        nc.tensor.matmul(gmm[:], wg2[:, gi, k, m * 128:(m + 1) * 128],
                                         h1g[:, k, :], start=(k == 0), stop=False)
                    nc.tensor.matmul(gmm[:], b2r[0:1, gi, m * 128:(m + 1) * 128],
                                     ones_w[0:1, :], start=False, stop=True)
                    nc.vector.scalar_tensor_tensor(
                        weighted[:, m, :], gmm[:], alB[:, l:l + 1],
                        sharedT[:, m, :], op0=AL.mult, op1=AL.add)
                nc.gpsimd.tensor_tensor(sh_acc[:], sh_acc[:], sharedT[:], op=AL.add)
                return weighted

            # =========================================================
            # software-pipelined main loop: iteration i runs the A-phase
            # (prep+MLPs) of stream i interleaved with the B-phase
            # (attention pool) of stream i-1, so the PE queue never sits
            # behind the pool's LN/softmax latency chains.
            # =========================================================
            groups = [[0, 1], [2, 3], [4, 5], [6, 7]]
            ar1_in = dr.tile([128, KH, T], f32, name="ar1_in")
            ar1_out = dr.tile([128, KH, T], f32, name="ar1_out")
            S = {}
            HS = {}
            for i in range(LPC + 1):
                bi = i - 1  # stream whose B-phase interleaves this iteration
                if i < LPC:
                    xcT = prep_stream(i)
                if bi >= 0:
                    pool_stats(bi, S[bi])
                if i < LPC:
                    h1s = mlp1(i, xcT, ws1, bs1, "s")
                if bi >= 0:
                    pool_qkv(bi, S[bi])
                if i < LPC:
                    h1g = mlp1(i, xcT, wg1[:, i // 6], bg1[:, i // 6], "g")
                    HS[i] = (h1s, h1g)
                if bi >= 1:
                    router(bi - 1, S[bi - 1]["w"])
                if bi >= 0:
                    pool_scores(bi, S[bi])
                if i < LPC:
                    S[i] = {"w": mlp2_weighted(i)}
                if bi >= 0:
                    pool_av(bi, S[bi])
                if i == LPC - 1:
                    # shared partial complete -> AllReduce #1 overlaps the
                    # remaining attention work
                    nc.sync.dma_start(ar1_in[:], sh_acc[:])
                    nc.gpsimd.collective_compute(
                        "AllReduce", AL.add, replica_groups=groups,
                        ins=[ar1_in.opt()], outs=[ar1_out.opt()])
                    nc.sync.dma_start(sh_acc[:], ar1_out[:])

            # time_ctx stream (index LPC): pool of shared_mean = sh_acc/24
            wsm = sp.tile([128, KH, T], bf16, name="wsm", tag="weighted", bufs=3)
            nc.scalar.activation(wsm[:], sh_acc[:], AF.Copy, scale=1.0 / L)
            S[LPC] = {"w": wsm}
            # last expert router + fusion AllReduce, overlapped with pool(12)
            router(LPC - 1, S[LPC - 1]["w"])
            ar2_in = dr.tile([128, KH, T], f32, name="ar2_in")
            ar2_out = dr.tile([128, KH, T], f32, name="ar2_out")
            nc.sync.dma_start(ar2_in[:], fus_acc[:])
            nc.gpsimd.collective_compute(
                "AllReduce", AL.add, replica_groups=groups,
                ins=[ar2_in.opt()], outs=[ar2_out.opt()])
            nc.sync.dma_start(fus_acc[:], ar2_out[:])
            pool_stats(LPC, S[LPC])
            pool_qkv(LPC, S[LPC])
            pool_scores(LPC, S[LPC])
            pool_av(LPC, S[LPC])

            # batched out_proj + time-mean for all 13 streams
            for m in range(KH):
                omm = ps.tile([128, NSTREAM], f32, name=f"tpb_{m}", tag="row")
                for k in range(KH):
                    nc.tensor.matmul(
                        omm[:], wo[:, k, m * 128:(m + 1) * 128],
                        osum_all[:, k * NSTREAM:(k + 1) * NSTREAM],
                        start=(k == 0), stop=(k == KH - 1))
                nc.scalar.activation(
                    tpT[:, m * NSTREAM:(m + 1) * NSTREAM], omm[:],
                    AF.Identity, scale=1.0 / T, bias=bo[:, m:m + 1])

            # ---------- fusion tail ----------
            fus_bf = sp.tile([128, KH, T], bf16, name="fus_bf", tag="sharedT", bufs=1)
            nc.scalar.activation(fus_bf[:], fus_acc[:], AF.Copy)
            fsq = sp.tile([128, KH, T], bf16, name="fsq", tag="sq", bufs=1)
            nc.vector.tensor_tensor(fsq[:], fus_bf[:], fus_bf[:], op=AL.mult)
            fs_ps = ps.tile([1, T], f32, name="fs_ps", tag="row")
            fq_ps = ps.tile([1, T], f32, name="fq_ps", tag="row")
            for c in range(KH):
                nc.tensor.matmul(fs_ps[:], ones_col[:], fus_bf[:, c, :],
                                 start=(c == 0), stop=(c == KH - 1))
            for c in range(KH):
                nc.tensor.matmul(fq_ps[:], ones_col[:], fsq[:, c, :],
                                 start=(c == 0), stop=(c == KH - 1))
            fmu = sp.tile([1, T], f32, name="fmu", tag="mu_row", bufs=2)
            nc.scalar.activation(fmu[:], fs_ps[:], AF.Copy, scale=1.0 / H)
            fe2 = sp.tile([1, T], f32, name="fe2", tag="e2_row", bufs=2)
            nc.scalar.activation(fe2[:], fq_ps[:], AF.Copy, scale=1.0 / H)
            fvar = sp.tile([1, T], f32, name="fvar", tag="var_row", bufs=2)
            nc.vector.tensor_tensor(fvar[:], fmu[:], fmu[:], op=AL.mult)
            nc.vector.tensor_tensor(fvar[:], fe2[:], fvar[:], op=AL.subtract)
            nc.scalar.activation(fvar[:], fvar[:], AF.Ln, bias=eps1[:])
            frs = sp.tile([1, T], f32, name="frs", tag="rstd_bf", bufs=2)
            nc.scalar.activation(frs[:], fvar[:], AF.Exp, scale=-0.5)
            fmB_ps = ps.tile([128, T], f32, name="fmB_ps", tag="mm400")
            nc.tensor.matmul(fmB_ps[:], ones_rowf[:], fmu[:], start=True, stop=True)
            fmB = sp.tile([128, T], f32, name="fmB", tag="invB")
            nc.any.tensor_copy(fmB[:], fmB_ps[:])
            frB_ps = ps.tile([128, T], f32, name="frB_ps", tag="mm400")
            nc.tensor.matmul(frB_ps[:], ones_rowf[:], frs[:], start=True, stop=True)
            frB = sp.tile([128, T], f32, name="frB", tag="frB")
            nc.any.tensor_copy(frB[:], frB_ps[:])
            fmean = sp.tile([128, KH], f32, name="fmean", tag="fmean", bufs=3)
            for c in range(KH):
                t1 = sp.tile([128, T], f32, name=f"ft1_{c}", tag="ft1")
                nc.vector.tensor_tensor(t1[:], fus_bf[:, c, :], fmB[:],
                                        op=AL.subtract)
                nc.vector.tensor_tensor(t1[:], t1[:], frB[:], op=AL.mult)
                nc.vector.tensor_scalar(t1[:], t1[:], scalar1=png[:, c:c + 1],
                                        scalar2=pnb[:, c:c + 1], op0=AL.mult,
                                        op1=AL.add)
                nc.vector.reduce_sum(fmean[:, c:c + 1], t1[:], axis=AX.X)
            nc.scalar.activation(fmean[:], fmean[:], AF.Copy, scale=1.0 / T)

            # fused_all = LN(fmean, pnf) + LN(time_ctx, pnf)
            tc_col = sp.tile([128, KH], f32, name="tc_col", tag="tc_col", bufs=3)
            nc.vector.tensor_copy(
                tc_col[:],
                tpT.rearrange("p (m s) -> p m s", s=NSTREAM)[:, :, LPC])
            lnf = ln_vec(fmean, pnfg, pnfb, "f")
            lnt = ln_vec(tc_col, pnfg, pnfb, "t")
            fa = sp.tile([128, KH], f32, name="fa", tag="fa", bufs=3)
            nc.vector.tensor_tensor(fa[:], lnf[:], lnt[:], op=AL.add)
            nc.sync.dma_start(fa_out_d[:], fa[:])

            # classifier (fp32, N=1)
            ch1 = sp.tile([128, 2], f32, name="ch1", tag="ch1", bufs=3)
            for m in range(2):
                cmm = ps.tile([128, 1], f32, name=f"c1_{m}", tag="row")
                for k in range(KH):
                    nc.tensor.matmul(cmm[:], cw1[:, k, m * 128:(m + 1) * 128],
                                     fa[:, k:k + 1], start=(k == 0),
                                     stop=(k == KH - 1))
                nc.scalar.activation(ch1[:, m:m + 1], cmm[:], AF.Gelu,
                                     bias=cb1[:, m:m + 1])
            ch2 = sp.tile([128, 1], f32, name="ch2", tag="ch2", bufs=3)
            cmm = ps.tile([128, 1], f32, name="c2", tag="row")
            for k in range(2):
                nc.tensor.matmul(cmm[:], cw2[:, k, :], ch1[:, k:k + 1],
                                 start=(k == 0), stop=(k == 1))
            nc.scalar.activation(ch2[:], cmm[:], AF.Gelu, bias=cb2[:])
            lgp = ps.tile([2, 1], f32, name="lgp", tag="row")
            nc.tensor.matmul(lgp[:], cw3[:], ch2[:], start=True, stop=True)
            lg_sb = sp.tile([2, 1], f32, name="lg_sb", tag="lg_sb", bufs=3)
            nc.vector.tensor_tensor(lg_sb[:], lgp[:], cb3[:], op=AL.add)
            nc.sync.dma_start(lg_out_d[:], lg_sb[:])

            # remaining outputs
            nc.sync.dma_start(rw_out_d[:], rw_row[:])
            tp_view = tp_out_d.rearrange("l (mo mi) -> mi mo l", mi=128)
            for mo in range(KH):
                nc.sync.dma_start(
                    tp_view[:, mo, :],
                    tpT[:, mo * NSTREAM: mo * NSTREAM + LPC])

    nc.compile()
    return nc


_NC_CACHE = {}


def _get_nc():
    if "nc" not in _NC_CACHE:
        _NC_CACHE["nc"] = _build_nc()
    return _NC_CACHE["nc"]


def _fold_weights(inp):
    """Host-side weight folding. Returns list of 8 per-core in_maps."""
    f32 = np.float32

    def c(a):
        return np.ascontiguousarray(a, dtype=f32)

    def kchunks(w):  # [K, M] -> [128, K//128, M]
        K, M = w.shape
        return np.ascontiguousarray(
            w.reshape(K // 128, 128, M).transpose(1, 0, 2))

    def cols(b):  # [M] -> [128, M//128]
        return np.ascontiguousarray(b.reshape(-1, 128).T)

    def bf(a):
        return np.ascontiguousarray(a.astype(_BF))

    x = c(inp["x"])
    ws1f = kchunks(c(inp["s_ln_g"])[:, None] * c(inp["s_w1"]))
    bs1f = cols(c(inp["s_ln_b"]) @ c(inp["s_w1"]) + c(inp["s_b1"]))
    ws2f = kchunks(c(inp["s_w2"]))
    bs2f = cols(c(inp["s_b2"]))

    p_in_w = c(inp["p_in_w"])
    p_ln_g, p_ln_b = c(inp["p_ln_g"]), c(inp["p_ln_b"])
    wqkf_full = p_ln_g[:, None] * p_in_w[:, :2 * H]
    bqkf = p_ln_b @ p_in_w[:, :2 * H] + c(inp["p_in_b"])[:2 * H]
    wvf_full = p_ln_g[:, None] * p_in_w[:, 2 * H:]
    bvf = p_ln_b @ p_in_w[:, 2 * H:] + c(inp["p_in_b"])[2 * H:]

    common = dict(
        ws1=bf(ws1f), bs1=bs1f, ws2=bf(ws2f), bs2=bs2f,
        wqk=bf(kchunks(wqkf_full)), bqk=cols(bqkf),
        wv=bf(kchunks(wvf_full)), bvr=bf(bvf[None, :]),
        wo=kchunks(c(inp["p_out_w"])), bo=cols(c(inp["p_out_b"])),
        rw1=kchunks((c(inp["p_out_w"]) @ c(inp["r_w1"])) / float(T)),
        rb1=(c(inp["r_w1"]).T @ c(inp["p_out_b"]) + c(inp["r_b1"]))[:, None],
        rw2=c(inp["r_w2"]),
        rb2t=c(inp["r_b2"]).reshape(1, 1) / (2.0 * TEMP),
        png=cols(c(inp["pn_g"])), pnb=cols(c(inp["pn_b"])),
        pnfg=cols(c(inp["pnf_g"])), pnfb=cols(c(inp["pnf_b"])),
        cw1=kchunks(c(inp["c_w1"])), cb1=cols(c(inp["c_b1"])),
        cw2=kchunks(c(inp["c_w2"])), cb2=cols(c(inp["c_b2"])),
        cw3=c(inp["c_w3"]), cb3=c(inp["c_b3"])[:, None],
    )

    g_ln_g, g_ln_b = c(inp["g_ln_g"]), c(inp["g_ln_b"])
    g_w1, g_b1 = c(inp["g_w1"]), c(inp["g_b1"])
    g_w2, g_b2 = c(inp["g_w2"]), c(inp["g_b2"])
    alpha = c(inp["mlp_alpha"])

    in_maps = []
    for core in range(NCORE):
        b, half = core // 2, core % 2
        gs = [2 * half, 2 * half + 1]
        # [2,128,KD,H] -> [128,2,KD,H]
        wg1c = np.stack(
            [kchunks(g_ln_g[g][:, None] * g_w1[g]) for g in gs]).transpose(1, 0, 2, 3)
        bg1c = np.stack(
            [cols(g_ln_b[g] @ g_w1[g] + g_b1[g]) for g in gs]).transpose(1, 0, 2)
        wg2c = np.stack([kchunks(g_w2[g]) for g in gs]).transpose(1, 0, 2, 3)
        b2rc = np.stack([g_b2[g] for g in gs])[None]  # [1,2,H]
        m = dict(common)
        m.update(
            x=np.ascontiguousarray(x[b, half * LPC:(half + 1) * LPC]),
            alpha=np.ascontiguousarray(alpha[half * LPC:(half + 1) * LPC]),
            wg1=bf(np.ascontiguousarray(wg1c)),
            bg1=np.ascontiguousarray(bg1c),
            wg2=bf(np.ascontiguousarray(wg2c)),
            b2r=bf(b2rc),
        )
        in_maps.append(m)
    return in_maps


def _run(inputs, trace=False):
    from concourse.bass_utils import run_bass_kernel_spmd

    inputs = {k: np.asarray(v) for k, v in inputs.items()}
    nc = _get_nc()
    in_maps = _fold_weights(inputs)
    res = run_bass_kernel_spmd(
        nc, in_maps, core_ids=list(range(NCORE)), trace=trace)
    outs = res.results

    logits = np.zeros((B, 2), np.float32)
    routing = np.zeros((B, L), np.float32)
    fused_all = np.zeros((B, H), np.float32)
    time_pooled = np.zeros((B, L, H), np.float32)
    for core in range(NCORE):
        b, half = core // 2, core % 2
        o = outs[core]
        sl = slice(half * LPC, (half + 1) * LPC)
        time_pooled[b, sl] = o["tp_out"]
        routing[b, sl] = o["rw_out"][0]
        if half == 0:
            logits[b] = o["lg_out"][:, 0]
            fused_all[b] = o["fa_out"].T.reshape(H)
    return (logits, routing, fused_all, time_pooled), res


def kernel(**inputs):
    return _run(inputs, trace=False)[0]
